# revision 10
# baseline (speedup 1.0000x reference)
"""Trainium2 Bass kernel for nn_Expert (gather-span + 2-layer linear MLP).

Reference computation (B=32, L=4096, H=1024, N=4):
    idx      = pos + arange(N)                      # (B, N)
    gathered = hidden[b, idx[b, n], :]              # (B, N, H)
    x        = gathered.reshape(B, N*H)             # (B, 4096)
    out      = (x @ W1.T + b1) @ W2.T + b2          # (B, 4)

Sharding (8 cores): hidden sharded on the last dim (H) in 128-wide
slices; W1 sharded over the matching contraction columns (2MB/core).
The kernel computes the per-core contraction partial of x @ W1.T
(a (32, 1024) fp32 tile); the host sums the 8 partials and applies
the tiny second layer (1024->4) plus both biases during the reduction
it already performs (both are linear, so this is exact).

Precision trick: fp32 matmuls on the PE take two passes per streamed
column (fp32_mode=LOW_HIGH, ~5.6 cyc/col); fp16 streams at 1 cyc/col.
W1 and x are each split into fp16 (hi, lo) parts and the product is
assembled from three fp16 passes accumulated exactly in fp32 PSUM:
    x@W1 ~ xh@wh + (xh*2^-11)@(wl*2^11) + xl@wh      (xl@wl ~ 2^-22, dropped)
The lo part of W1 (~W1*2^-11 ~ 7e-6) would be fp16-subnormal, so the
host pre-scales it by 2^11 and the matching stationary is down-scaled
on device; all operands stay in fp16 normal range and all three
passes accumulate at natural scale. Verified ~1.2e-4 max rel err.

Device-side critical path:
  1. gather indices idx[b] = b*L + pos[b] host-computed, shipped as a
     direct (32, 1) int32 DMA, first on the sync queue (its 32 tiny
     packets drain before the W1 tiles queue behind it),
  2. indirect gather pulls one 2KB span (4 consecutive 512B rows) per
     batch -> xg (32, 512),
  3. dummy PE transposes keep the HAM activity monitor busy so the
     clock is at 2.4 GHz (not the cold 1.2 GHz) when real work lands,
  4. 4 PE strip transposes -> xT (128, 4x32 strips in one PSUM bank),
     then DVE/ACT build the fp16 hi/lo/scaled stationaries,
  5. stage 1 on PE in half-major order: 12 fp16 matmuls per PSUM half
     so half 0's copy-out + 64KB store overlap half 1's matmuls.
"""

import numpy as np

from concourse import bass, bacc, mybir
from concourse.tile import TileContext
from concourse.bass_utils import run_bass_kernel_spmd
from concourse.masks import make_identity

B, L, H, N = 32, 4096, 1024, 4
NCORES = 8
HS = H // NCORES       # 128: per-core slice of the hidden dim
P = 128
HB = H // 2            # 512: psum bank width for stage 1
F32 = mybir.dt.float32
F16 = mybir.dt.float16
I32 = mybir.dt.int32
NWARM = 20             # dummy PE ops to hold the clock at 2.4 GHz
LO_SCALE = 2048.0      # 2^11: host upscale of W1's lo part

TRACE = False          # set True in test harnesses to profile
LAST_EXEC_NS = None

_nc_cache = None


def _build_nc():
    nc = bacc.Bacc(target_bir_lowering=False)
    hid = nc.declare_dram_parameter("hid", [B * L, HS], F32, isOutput=False)
    idxd = nc.declare_dram_parameter("idxd", [B, 1], I32, isOutput=False)
    w1h = nc.declare_dram_parameter("w1h", [N * P, H], F16, isOutput=False)
    w1l = nc.declare_dram_parameter("w1l", [N * P, H], F16, isOutput=False)
    out = nc.declare_dram_parameter("out", [B, H], F32, isOutput=True)

    with TileContext(nc) as tc:
        with (
            tc.tile_pool(name="sbuf", bufs=1) as spool,
            tc.tile_pool(name="ps1", bufs=2, space="PSUM") as ppool,
            tc.tile_pool(name="psx", bufs=1, space="PSUM") as xpool,
        ):
            # gather indices: direct (32, 1) int32 DMA on the SWDGE path
            # (GpSimd issues it and waits its own queue's completion, so
            # the gather's descriptor-gen starts without the cross-engine
            # HWDGE completion round trip)
            idx = spool.tile([B, 1], I32)
            nc.gpsimd.dma_start(out=idx[:], in_=idxd[:])

            # W1 hi/lo tiles (fp16) split across both HWDGE queues in
            # consumption order: hi tiles stream first
            whsb, wlsb = [], []
            for n in range(N):
                t = spool.tile([P, H], F16, tag=f"wh_{n}", name=f"wh_{n}")
                eng = nc.sync if n < 2 else nc.scalar
                eng.dma_start(out=t[:], in_=w1h[n * P:(n + 1) * P, :])
                whsb.append(t)
            for n in range(N):
                t = spool.tile([P, H], F16, tag=f"wl_{n}", name=f"wl_{n}")
                eng = nc.sync if n < 2 else nc.scalar
                eng.dma_start(out=t[:], in_=w1l[n * P:(n + 1) * P, :])
                wlsb.append(t)

            ident = spool.tile([P, P], F32)
            make_identity(nc, ident[:])
            # constant for down-scaling the hi stationary by 2^-11
            cscale = spool.tile([P, B], F32)
            nc.vector.memset(cscale[:], 1.0 / LO_SCALE)

            # indirect gather: xg[b, n*128+k] = hidden[b, pos[b]+n, k]
            # (one 2KB descriptor per batch: 4 consecutive rows of hid)
            xg = spool.tile([B, N * HS], F32)
            nc.gpsimd.indirect_dma_start(
                out=xg[:, :],
                out_offset=None,
                in_=hid[:],
                in_offset=bass.IndirectOffsetOnAxis(ap=idx[:, :1], axis=0),
                bounds_check=None,
            )

            # PE warmup: independent dummy transposes that the scheduler
            # front-loads; they span the gather wait so the HAM window
            # flips to 2.4 GHz and stays there
            warm_ps = xpool.tile([P, P], F32, space="PSUM", tag="warm")
            for _ in range(NWARM):
                nc.tensor.transpose(
                    out=warm_ps[:], in_=ident[:], identity=ident[:]
                )

            # 4 strip transposes into one shared PSUM tile:
            # xt_ps[k, n*32+b] = xg[b, n*128+k]
            xt_ps = xpool.tile([P, P], F32, space="PSUM", tag="xt")
            for n in range(N):
                nc.tensor.transpose(
                    out=xt_ps[:, n * B:(n + 1) * B],
                    in_=xg[:, n * HS:(n + 1) * HS],
                    identity=ident[:B, :B],
                )

            # fp16 stationaries per strip: hi, hi*2^-11, lo = x - hi.
            # Grouped by pass (not by strip) so pass-1 stationaries all
            # exist before the first matmul wants them; the hi->fp32
            # round trip for the lo part runs on ACT in parallel.
            xh, xhs, xl = [], [], []
            for n in range(N):
                h16 = spool.tile([P, B], F16, tag=f"xh{n}")
                nc.vector.tensor_copy(
                    out=h16[:], in_=xt_ps[:, n * B:(n + 1) * B]
                )
                xh.append(h16)
            h32s = []
            for n in range(N):
                h32 = spool.tile([P, B], F32, tag=f"xh32{n}")
                nc.scalar.copy(out=h32[:], in_=xh[n][:])
                h32s.append(h32)
            for n in range(N):
                hs16 = spool.tile([P, B], F16, tag=f"xhs{n}")
                nc.vector.tensor_tensor(
                    out=hs16[:], in0=xt_ps[:, n * B:(n + 1) * B],
                    in1=cscale[:], op=mybir.AluOpType.mult,
                )
                xhs.append(hs16)
            for n in range(N):
                l16 = spool.tile([P, B], F16, tag=f"xl{n}")
                nc.vector.tensor_tensor(
                    out=l16[:], in0=xt_ps[:, n * B:(n + 1) * B],
                    in1=h32s[n][:], op=mybir.AluOpType.subtract,
                )
                xl.append(l16)

            # stage 1, half-major: 3 fp16 passes x 4 chunks per half
            ps = [
                ppool.tile([B, HB], F32, space="PSUM", tag="ps1",
                           name=f"ps1_{i}")
                for i in range(2)
            ]
            passes = [(xh, whsb), (xhs, wlsb), (xl, whsb)]
            osb = spool.tile([B, H], F32)
            for half in range(2):
                for p, (stat, stream) in enumerate(passes):
                    for n in range(N):
                        nc.tensor.matmul(
                            out=ps[half][:],
                            lhsT=stat[n][:],
                            rhs=stream[n][:, half * HB:(half + 1) * HB],
                            start=(p == 0 and n == 0),
                            stop=(p == 2 and n == N - 1),
                        )
                if half == 0:
                    nc.vector.tensor_copy(out=osb[:, :HB], in_=ps[0][:])
                    nc.sync.dma_start(out=out[:, :HB], in_=osb[:, :HB])
                else:
                    # final half: split across two engines + two queues so
                    # the last copy/store pair is half-sized
                    HQ = HB // 2
                    nc.vector.tensor_copy(
                        out=osb[:, HB:HB + HQ], in_=ps[1][:, :HQ]
                    )
                    nc.sync.dma_start(
                        out=out[:, HB:HB + HQ], in_=osb[:, HB:HB + HQ]
                    )
                    nc.scalar.copy(out=osb[:, HB + HQ:], in_=ps[1][:, HQ:])
                    nc.scalar.dma_start(
                        out=out[:, HB + HQ:], in_=osb[:, HB + HQ:]
                    )

    nc.finalize()
    return nc


def _get_nc():
    global _nc_cache
    if _nc_cache is None:
        _nc_cache = _build_nc()
    return _nc_cache


def kernel(hidden, pos, W1, b1, W2, b2):
    global LAST_EXEC_NS
    hidden = np.asarray(hidden, dtype=np.float32)
    pos = np.asarray(pos)
    W1 = np.asarray(W1, dtype=np.float32)
    b1 = np.asarray(b1, dtype=np.float32)
    W2 = np.asarray(W2, dtype=np.float32)
    b2 = np.asarray(b2, dtype=np.float32)

    # gather row indices into hid (B*L, HS): idx[b] = b*L + pos[b]
    posv = pos.reshape(B).astype(np.int64)
    idxd = (np.arange(B, dtype=np.int64) * L + posv).reshape(B, 1).astype(
        np.int32
    )

    # W1 (H, N*H) -> per-core (N*P, H) fp16 hi/lo:
    #   w1t_j[n*P+k, o] = W1[o, n*H+j*HS+k]
    w1r = W1.reshape(H, N, NCORES, HS)                 # [o, n, j, k]

    in_maps = []
    for j in range(NCORES):
        hid_j = np.ascontiguousarray(
            hidden[:, :, j * HS:(j + 1) * HS]
        ).reshape(B * L, HS)
        w1t_j = np.ascontiguousarray(
            w1r[:, :, j, :].transpose(1, 2, 0).reshape(N * P, H)
        )
        w1h_j = w1t_j.astype(np.float16)
        w1l_j = ((w1t_j - w1h_j.astype(np.float32)) * LO_SCALE).astype(
            np.float16
        )
        in_maps.append(
            {"hid": hid_j, "idxd": idxd, "w1h": w1h_j, "w1l": w1l_j}
        )

    nc = _get_nc()
    res = run_bass_kernel_spmd(nc, in_maps, list(range(NCORES)), trace=TRACE)
    LAST_EXEC_NS = res.exec_time_ns

    parts = np.stack([res.results[j]["out"] for j in range(NCORES)])  # (8,32,1024)
    out1 = parts.sum(axis=0, dtype=np.float64) + b1.astype(np.float64)
    y = out1 @ W2.T.astype(np.float64) + b2.astype(np.float64)
    return np.ascontiguousarray(y.astype(np.float32))                 # (B, N)


# revision 13
# speedup vs baseline: 1.0016x; 1.0016x over previous
"""Trainium2 Bass kernel for nn_Expert (gather-span + 2-layer linear MLP).

Reference computation (B=32, L=4096, H=1024, N=4):
    idx      = pos + arange(N)                      # (B, N)
    gathered = hidden[b, idx[b, n], :]              # (B, N, H)
    x        = gathered.reshape(B, N*H)             # (B, 4096)
    out      = (x @ W1.T + b1) @ W2.T + b2          # (B, 4)

Sharding (8 cores): hidden sharded on the last dim (H) in 128-wide
slices; W1 sharded over the matching contraction columns (2MB/core).
The kernel computes the per-core contraction partial of x @ W1.T
(a (32, 1024) fp32 tile); the host sums the 8 partials and applies
the tiny second layer (1024->4) plus both biases during the reduction
it already performs (both are linear, so this is exact).

Precision trick: fp32 matmuls on the PE take two passes per streamed
column (fp32_mode=LOW_HIGH, ~5.6 cyc/col); fp16 streams at 1 cyc/col.
W1 and x are each split into fp16 (hi, lo) parts and the product is
assembled from three fp16 passes accumulated exactly in fp32 PSUM:
    x@W1 ~ xh@wh + (xh*2^-11)@(wl*2^11) + xl@wh      (xl@wl ~ 2^-22, dropped)
The lo part of W1 (~W1*2^-11 ~ 7e-6) would be fp16-subnormal, so the
host pre-scales it by 2^11 and the matching stationary is down-scaled
on device; all operands stay in fp16 normal range and all three
passes accumulate at natural scale. Verified ~1.2e-4 max rel err.

Device-side critical path:
  1. gather indices idx[b] = b*L + pos[b] host-computed, shipped as a
     direct (32, 1) int32 DMA, first on the sync queue (its 32 tiny
     packets drain before the W1 tiles queue behind it),
  2. indirect gather pulls one 2KB span (4 consecutive 512B rows) per
     batch -> xg (32, 512),
  3. dummy PE transposes keep the HAM activity monitor busy so the
     clock is at 2.4 GHz (not the cold 1.2 GHz) when real work lands,
  4. 4 PE strip transposes -> xT (128, 4x32 strips in one PSUM bank),
     then DVE/ACT build the fp16 hi/lo/scaled stationaries,
  5. stage 1 on PE in half-major order: 12 fp16 matmuls per PSUM half
     so half 0's copy-out + 64KB store overlap half 1's matmuls.
"""

import numpy as np

from concourse import bass, bacc, mybir
from concourse.tile import TileContext
from concourse.bass_utils import run_bass_kernel_spmd
from concourse.masks import make_identity

B, L, H, N = 32, 4096, 1024, 4
NCORES = 8
HS = H // NCORES       # 128: per-core slice of the hidden dim
P = 128
HB = H // 2            # 512: psum bank width for stage 1
F32 = mybir.dt.float32
F16 = mybir.dt.float16
I32 = mybir.dt.int32
NWARM = 24             # dummy PE ops to hold the clock at 2.4 GHz
LO_SCALE = 2048.0      # 2^11: host upscale of W1's lo part

TRACE = False          # set True in test harnesses to profile
LAST_EXEC_NS = None

_nc_cache = None


def _build_nc():
    nc = bacc.Bacc(target_bir_lowering=False)
    hid = nc.declare_dram_parameter("hid", [B * L, HS], F32, isOutput=False)
    idxd = nc.declare_dram_parameter("idxd", [B, 1], I32, isOutput=False)
    w1h = nc.declare_dram_parameter("w1h", [N * P, H], F16, isOutput=False)
    w1l = nc.declare_dram_parameter("w1l", [N * P, H], F16, isOutput=False)
    out = nc.declare_dram_parameter("out", [B, H], F32, isOutput=True)

    with TileContext(nc) as tc:
        with (
            tc.tile_pool(name="sbuf", bufs=1) as spool,
            tc.tile_pool(name="ps1", bufs=2, space="PSUM") as ppool,
            tc.tile_pool(name="psx", bufs=1, space="PSUM") as xpool,
        ):
            # gather indices: direct (32, 1) int32 DMA, first on the sync
            # queue (HWDGE; measured faster end-to-end than the SWDGE path)
            idx = spool.tile([B, 1], I32)
            nc.sync.dma_start(out=idx[:], in_=idxd[:])

            # W1 hi/lo tiles (fp16) split across both HWDGE queues in
            # consumption order: hi tiles stream first
            whsb, wlsb = [], []
            for n in range(N):
                t = spool.tile([P, H], F16, tag=f"wh_{n}", name=f"wh_{n}")
                eng = nc.sync if n < 2 else nc.scalar
                eng.dma_start(out=t[:], in_=w1h[n * P:(n + 1) * P, :])
                whsb.append(t)
            for n in range(N):
                t = spool.tile([P, H], F16, tag=f"wl_{n}", name=f"wl_{n}")
                eng = nc.sync if n < 2 else nc.scalar
                eng.dma_start(out=t[:], in_=w1l[n * P:(n + 1) * P, :])
                wlsb.append(t)

            ident = spool.tile([P, P], F32)
            make_identity(nc, ident[:])
            # constant for down-scaling the hi stationary by 2^-11
            cscale = spool.tile([P, B], F32)
            nc.vector.memset(cscale[:], 1.0 / LO_SCALE)

            # indirect gather: xg[b, n*128+k] = hidden[b, pos[b]+n, k]
            # (one 2KB descriptor per batch: 4 consecutive rows of hid)
            xg = spool.tile([B, N * HS], F32)
            nc.gpsimd.indirect_dma_start(
                out=xg[:, :],
                out_offset=None,
                in_=hid[:],
                in_offset=bass.IndirectOffsetOnAxis(ap=idx[:, :1], axis=0),
                bounds_check=None,
            )

            # PE warmup: independent dummy transposes that the scheduler
            # front-loads; they span the gather wait so the HAM window
            # flips to 2.4 GHz and stays there
            warm_ps = xpool.tile([P, P], F32, space="PSUM", tag="warm")
            for _ in range(NWARM):
                nc.tensor.transpose(
                    out=warm_ps[:], in_=ident[:], identity=ident[:]
                )

            # 4 strip transposes into one shared PSUM tile:
            # xt_ps[k, n*32+b] = xg[b, n*128+k]
            xt_ps = xpool.tile([P, P], F32, space="PSUM", tag="xt")
            for n in range(N):
                nc.tensor.transpose(
                    out=xt_ps[:, n * B:(n + 1) * B],
                    in_=xg[:, n * HS:(n + 1) * HS],
                    identity=ident[:B, :B],
                )

            # fp16 stationaries per strip: hi, hi*2^-11, lo = x - hi.
            # Grouped by pass (not by strip) so pass-1 stationaries all
            # exist before the first matmul wants them; the hi->fp32
            # round trip for the lo part runs on ACT in parallel.
            xh, xhs, xl = [], [], []
            for n in range(N):
                h16 = spool.tile([P, B], F16, tag=f"xh{n}")
                nc.vector.tensor_copy(
                    out=h16[:], in_=xt_ps[:, n * B:(n + 1) * B]
                )
                xh.append(h16)
            h32s = []
            for n in range(N):
                h32 = spool.tile([P, B], F32, tag=f"xh32{n}")
                nc.scalar.copy(out=h32[:], in_=xh[n][:])
                h32s.append(h32)
            for n in range(N):
                hs16 = spool.tile([P, B], F16, tag=f"xhs{n}")
                nc.vector.tensor_tensor(
                    out=hs16[:], in0=xt_ps[:, n * B:(n + 1) * B],
                    in1=cscale[:], op=mybir.AluOpType.mult,
                )
                xhs.append(hs16)
            for n in range(N):
                l16 = spool.tile([P, B], F16, tag=f"xl{n}")
                nc.vector.tensor_tensor(
                    out=l16[:], in0=xt_ps[:, n * B:(n + 1) * B],
                    in1=h32s[n][:], op=mybir.AluOpType.subtract,
                )
                xl.append(l16)

            # stage 1, half-major: 3 fp16 passes x 4 chunks per half
            ps = [
                ppool.tile([B, HB], F32, space="PSUM", tag="ps1",
                           name=f"ps1_{i}")
                for i in range(2)
            ]
            passes = [(xh, whsb), (xhs, wlsb), (xl, whsb)]
            osb = spool.tile([B, H], F32)
            for half in range(2):
                for p, (stat, stream) in enumerate(passes):
                    for n in range(N):
                        nc.tensor.matmul(
                            out=ps[half][:],
                            lhsT=stat[n][:],
                            rhs=stream[n][:, half * HB:(half + 1) * HB],
                            start=(p == 0 and n == 0),
                            stop=(p == 2 and n == N - 1),
                        )
                if half == 0:
                    nc.vector.tensor_copy(out=osb[:, :HB], in_=ps[0][:])
                    nc.sync.dma_start(out=out[:, :HB], in_=osb[:, :HB])
                else:
                    # final half: 3-way split across engines + queues so
                    # the very last copy/store pair is quarter-sized
                    HQ = HB // 2   # 256
                    HE = HB // 4   # 128
                    nc.vector.tensor_copy(
                        out=osb[:, HB:HB + HQ], in_=ps[1][:, :HQ]
                    )
                    nc.sync.dma_start(
                        out=out[:, HB:HB + HQ], in_=osb[:, HB:HB + HQ]
                    )
                    nc.scalar.copy(
                        out=osb[:, HB + HQ:HB + HQ + HE],
                        in_=ps[1][:, HQ:HQ + HE],
                    )
                    nc.scalar.dma_start(
                        out=out[:, HB + HQ:HB + HQ + HE],
                        in_=osb[:, HB + HQ:HB + HQ + HE],
                    )
                    nc.vector.tensor_copy(
                        out=osb[:, HB + HQ + HE:], in_=ps[1][:, HQ + HE:]
                    )
                    nc.sync.dma_start(
                        out=out[:, HB + HQ + HE:], in_=osb[:, HB + HQ + HE:]
                    )

    nc.finalize()
    return nc


def _get_nc():
    global _nc_cache
    if _nc_cache is None:
        _nc_cache = _build_nc()
    return _nc_cache


def kernel(hidden, pos, W1, b1, W2, b2):
    global LAST_EXEC_NS
    hidden = np.asarray(hidden, dtype=np.float32)
    pos = np.asarray(pos)
    W1 = np.asarray(W1, dtype=np.float32)
    b1 = np.asarray(b1, dtype=np.float32)
    W2 = np.asarray(W2, dtype=np.float32)
    b2 = np.asarray(b2, dtype=np.float32)

    # gather row indices into hid (B*L, HS): idx[b] = b*L + pos[b]
    posv = pos.reshape(B).astype(np.int64)
    idxd = (np.arange(B, dtype=np.int64) * L + posv).reshape(B, 1).astype(
        np.int32
    )

    # W1 (H, N*H) -> per-core (N*P, H) fp16 hi/lo:
    #   w1t_j[n*P+k, o] = W1[o, n*H+j*HS+k]
    w1r = W1.reshape(H, N, NCORES, HS)                 # [o, n, j, k]

    in_maps = []
    for j in range(NCORES):
        hid_j = np.ascontiguousarray(
            hidden[:, :, j * HS:(j + 1) * HS]
        ).reshape(B * L, HS)
        w1t_j = np.ascontiguousarray(
            w1r[:, :, j, :].transpose(1, 2, 0).reshape(N * P, H)
        )
        w1h_j = w1t_j.astype(np.float16)
        w1l_j = ((w1t_j - w1h_j.astype(np.float32)) * LO_SCALE).astype(
            np.float16
        )
        in_maps.append(
            {"hid": hid_j, "idxd": idxd, "w1h": w1h_j, "w1l": w1l_j}
        )

    nc = _get_nc()
    res = run_bass_kernel_spmd(nc, in_maps, list(range(NCORES)), trace=TRACE)
    LAST_EXEC_NS = res.exec_time_ns

    parts = np.stack([res.results[j]["out"] for j in range(NCORES)])  # (8,32,1024)
    out1 = parts.sum(axis=0, dtype=np.float64) + b1.astype(np.float64)
    y = out1 @ W2.T.astype(np.float64) + b2.astype(np.float64)
    return np.ascontiguousarray(y.astype(np.float32))                 # (B, N)


# revision 15
# speedup vs baseline: 1.0322x; 1.0305x over previous
"""Trainium2 Bass kernel for nn_Expert (gather-span + 2-layer linear MLP).

Reference computation (B=32, L=4096, H=1024, N=4):
    idx      = pos + arange(N)                      # (B, N)
    gathered = hidden[b, idx[b, n], :]              # (B, N, H)
    x        = gathered.reshape(B, N*H)             # (B, 4096)
    out      = (x @ W1.T + b1) @ W2.T + b2          # (B, 4)

Sharding (8 cores): hidden sharded on the last dim (H) in 128-wide
slices; W1 sharded over the matching contraction columns (2MB/core).
The kernel computes the per-core contraction partial of x @ W1.T
(a (32, 1024) fp32 tile); the host sums the 8 partials and applies
the tiny second layer (1024->4) plus both biases during the reduction
it already performs (both are linear, so this is exact).

Precision trick: fp32 matmuls on the PE take two passes per streamed
column (~5.6 cyc/col); fp16 streams at 1 cyc/col. W1 and x are each
split into fp16 (hi, lo) parts and the product is assembled from
three fp16 passes accumulated exactly in fp32 PSUM:
    x@W1 ~ xh@wh + xl@wh + (xh*2^-11)@(wl*2^11)     (xl@wl ~ 2^-22, dropped)
W1's lo residuals (~7e-6) would be fp16-subnormal, so the host
pre-scales them by 2^11 and the matching hi stationary is down-scaled
on device; every operand stays in fp16 normal range and all passes
accumulate at natural scale. Measured ~7e-5 max rel err.

Latency engineering (the kernel is dominated by fixed latencies):
  - gather indices idx[b] = b*L + pos[b] are host-computed, shipped as
    a direct (32, 1) int32 DMA, first on the sync queue,
  - only W1's hi half (1MB) streams before the gather so HBM is quiet
    while the indirect gather's 32 2KB-span descriptors are in flight;
    the lo half is released only after the gather data lands (a 1-elem
    copy from xg into each lo tile forces the ordering), and the
    wl-consuming pass runs last so the late start is hidden,
  - the PE runs dummy transposes/matmuls sized to span the gather wait
    so the HAM activity monitor holds the clock at 2.4 GHz (a cold PE
    runs at 1.2 GHz and needs ~3.4us of sustained work to ramp),
  - stage 1 runs half-major (12 matmuls per PSUM half) so half 0's
    copy-out + store overlap half 1's matmuls; the final half's
    copy/store is split 3 ways across engines and queues.
"""

import numpy as np

from concourse import bass, bacc, mybir
from concourse.tile import TileContext
from concourse.bass_utils import run_bass_kernel_spmd
from concourse.masks import make_identity

B, L, H, N = 32, 4096, 1024, 4
NCORES = 8
HS = H // NCORES       # 128: per-core slice of the hidden dim
P = 128
HB = H // 2            # 512: psum bank width for stage 1
F32 = mybir.dt.float32
F16 = mybir.dt.float16
I32 = mybir.dt.int32
NWARM32 = 8            # cold fp32 dummy transposes (~0.42us each)
NWARM16 = 13           # warm fp16 dummy matmuls (~0.22us each)
LO_SCALE = 2048.0      # 2^11: host upscale of W1's lo part

TRACE = False          # set True in test harnesses to profile
LAST_EXEC_NS = None

_nc_cache = None


def _build_nc():
    nc = bacc.Bacc(target_bir_lowering=False)
    hid = nc.declare_dram_parameter("hid", [B * L, HS], F32, isOutput=False)
    idxd = nc.declare_dram_parameter("idxd", [B, 1], I32, isOutput=False)
    # hi/lo W1, each as two (128, 2048) tiles: w1h_a = chunks 0,1 etc.
    w1ha = nc.declare_dram_parameter("w1ha", [P, 2 * H], F16, isOutput=False)
    w1hb = nc.declare_dram_parameter("w1hb", [P, 2 * H], F16, isOutput=False)
    w1la = nc.declare_dram_parameter("w1la", [P, 2 * H], F16, isOutput=False)
    w1lb = nc.declare_dram_parameter("w1lb", [P, 2 * H], F16, isOutput=False)
    out = nc.declare_dram_parameter("out", [B, H], F32, isOutput=True)

    with TileContext(nc) as tc:
        with (
            tc.tile_pool(name="sbuf", bufs=1) as spool,
            tc.tile_pool(name="ps1", bufs=2, space="PSUM") as ppool,
            tc.tile_pool(name="psx", bufs=1, space="PSUM") as xpool,
        ):
            # gather indices: direct (32, 1) int32 DMA, first on sync
            idx = spool.tile([B, 1], I32)
            nc.sync.dma_start(out=idx[:], in_=idxd[:])

            # W1 hi tiles stream immediately (done before gather flight)
            wha = spool.tile([P, 2 * H], F16, tag="wha", name="wha")
            nc.sync.dma_start(out=wha[:], in_=w1ha[:])
            whb = spool.tile([P, 2 * H], F16, tag="whb", name="whb")
            nc.scalar.dma_start(out=whb[:], in_=w1hb[:])

            ident = spool.tile([P, P], F32)
            make_identity(nc, ident[:])
            dummy16 = spool.tile([P, B], F16)
            nc.vector.memset(dummy16[:], 1.0)

            # indirect gather: xg[b, n*128+k] = hidden[b, pos[b]+n, k]
            # (one 2KB descriptor per batch: 4 consecutive rows of hid)
            xg = spool.tile([B, N * HS], F32)
            nc.gpsimd.indirect_dma_start(
                out=xg[:, :],
                out_offset=None,
                in_=hid[:],
                in_offset=bass.IndirectOffsetOnAxis(ap=idx[:, :1], axis=0),
                bounds_check=None,
            )

            # W1 lo tiles are released only after the gather data lands
            # (keeps HBM quiet during the gather's scattered reads); the
            # 1-elem copy from xg forces the ordering
            wla = spool.tile([P, 2 * H], F16, tag="wla", name="wla")
            nc.vector.tensor_copy(out=wla[:1, :2], in_=xg[:1, :2])
            nc.sync.dma_start(out=wla[:], in_=w1la[:])
            wlb = spool.tile([P, 2 * H], F16, tag="wlb", name="wlb")
            nc.vector.tensor_copy(out=wlb[:1, :2], in_=xg[:1, :2])
            nc.scalar.dma_start(out=wlb[:], in_=w1lb[:])

            def wh(n):
                t = wha if n < 2 else whb
                return t[:, (n % 2) * H:(n % 2) * H + H]

            def wl(n):
                t = wla if n < 2 else wlb
                return t[:, (n % 2) * H:(n % 2) * H + H]

            # PE warmup: cold fp32 transposes, then fp16 dummy matmuls
            # streaming the landed hi tile; spans the gather wait so the
            # HAM window stays hot into the real matmuls
            warm_ps = xpool.tile([P, P], F32, space="PSUM", tag="warm")
            for _ in range(NWARM32):
                nc.tensor.transpose(
                    out=warm_ps[:], in_=ident[:], identity=ident[:]
                )
            warm2_ps = xpool.tile([B, HB], F32, space="PSUM", tag="warm2")
            for _ in range(NWARM16):
                nc.tensor.matmul(
                    out=warm2_ps[:], lhsT=dummy16[:],
                    rhs=wha[:, :HB], start=True, stop=True,
                )

            # 4 strip transposes into one shared PSUM tile:
            # xt_ps[k, n*32+b] = xg[b, n*128+k]
            xt_ps = xpool.tile([P, P], F32, space="PSUM", tag="xt")
            for n in range(N):
                nc.tensor.transpose(
                    out=xt_ps[:, n * B:(n + 1) * B],
                    in_=xg[:, n * HS:(n + 1) * HS],
                    identity=ident[:B, :B],
                )

            # fp16 stationaries per strip: hi (DVE cast), hi*2^-11 (ACT
            # scaled copy), lo = x - hi (ACT fp32 round trip + DVE sub)
            xh, xhs, xl = [], [], []
            for n in range(N):
                h16 = spool.tile([P, B], F16, tag=f"xh{n}")
                nc.vector.tensor_copy(
                    out=h16[:], in_=xt_ps[:, n * B:(n + 1) * B]
                )
                xh.append(h16)
            h32s = []
            for n in range(N):
                h32 = spool.tile([P, B], F32, tag=f"xh32{n}")
                nc.scalar.copy(out=h32[:], in_=xh[n][:])
                h32s.append(h32)
            for n in range(N):
                l16 = spool.tile([P, B], F16, tag=f"xl{n}")
                nc.vector.tensor_tensor(
                    out=l16[:], in0=xt_ps[:, n * B:(n + 1) * B],
                    in1=h32s[n][:], op=mybir.AluOpType.subtract,
                )
                xl.append(l16)
            for n in range(N):
                hs16 = spool.tile([P, B], F16, tag=f"xhs{n}")
                nc.scalar.activation(
                    out=hs16[:], in_=xt_ps[:, n * B:(n + 1) * B],
                    func=mybir.ActivationFunctionType.Copy,
                    scale=1.0 / LO_SCALE,
                )
                xhs.append(hs16)

            # stage 1, half-major; the wl-consuming pass runs last so the
            # late-released lo tiles are off the critical path
            ps = [
                ppool.tile([B, HB], F32, space="PSUM", tag="ps1",
                           name=f"ps1_{i}")
                for i in range(2)
            ]
            passes = [(xh, wh), (xl, wh), (xhs, wl)]
            osb = spool.tile([B, H], F32)
            for half in range(2):
                for p, (stat, stream) in enumerate(passes):
                    for n in range(N):
                        nc.tensor.matmul(
                            out=ps[half][:],
                            lhsT=stat[n][:],
                            rhs=stream(n)[:, half * HB:(half + 1) * HB],
                            start=(p == 0 and n == 0),
                            stop=(p == 2 and n == N - 1),
                        )
                if half == 0:
                    nc.vector.tensor_copy(out=osb[:, :HB], in_=ps[0][:])
                    nc.sync.dma_start(out=out[:, :HB], in_=osb[:, :HB])
                else:
                    # final half: 3-way split across engines + queues so
                    # the very last copy/store pair is quarter-sized
                    HQ = HB // 2   # 256
                    HE = HB // 4   # 128
                    nc.vector.tensor_copy(
                        out=osb[:, HB:HB + HQ], in_=ps[1][:, :HQ]
                    )
                    nc.sync.dma_start(
                        out=out[:, HB:HB + HQ], in_=osb[:, HB:HB + HQ]
                    )
                    nc.scalar.copy(
                        out=osb[:, HB + HQ:HB + HQ + HE],
                        in_=ps[1][:, HQ:HQ + HE],
                    )
                    nc.scalar.dma_start(
                        out=out[:, HB + HQ:HB + HQ + HE],
                        in_=osb[:, HB + HQ:HB + HQ + HE],
                    )
                    nc.vector.tensor_copy(
                        out=osb[:, HB + HQ + HE:], in_=ps[1][:, HQ + HE:]
                    )
                    nc.sync.dma_start(
                        out=out[:, HB + HQ + HE:], in_=osb[:, HB + HQ + HE:]
                    )

    nc.finalize()
    return nc


def _get_nc():
    global _nc_cache
    if _nc_cache is None:
        _nc_cache = _build_nc()
    return _nc_cache


def kernel(hidden, pos, W1, b1, W2, b2):
    global LAST_EXEC_NS
    hidden = np.asarray(hidden, dtype=np.float32)
    pos = np.asarray(pos)
    W1 = np.asarray(W1, dtype=np.float32)
    b1 = np.asarray(b1, dtype=np.float32)
    W2 = np.asarray(W2, dtype=np.float32)
    b2 = np.asarray(b2, dtype=np.float32)

    # gather row indices into hid (B*L, HS): idx[b] = b*L + pos[b]
    posv = pos.reshape(B).astype(np.int64)
    idxd = (np.arange(B, dtype=np.int64) * L + posv).reshape(B, 1).astype(
        np.int32
    )

    # W1 (H, N*H) -> per-core (N*P, H) fp16 hi/lo:
    #   w1t_j[n*P+k, o] = W1[o, n*H+j*HS+k]
    w1r = W1.reshape(H, N, NCORES, HS)                 # [o, n, j, k]

    in_maps = []
    for j in range(NCORES):
        hid_j = np.ascontiguousarray(
            hidden[:, :, j * HS:(j + 1) * HS]
        ).reshape(B * L, HS)
        w1t_j = w1r[:, :, j, :].transpose(1, 2, 0)     # [n, k, o]
        w1h_j = w1t_j.astype(np.float16)
        w1l_j = ((w1t_j - w1h_j.astype(np.float32)) * LO_SCALE).astype(
            np.float16
        )
        # (n, k, o) -> tiles (k, n*H+o) with n in {0,1} / {2,3}
        w1ha_j = np.ascontiguousarray(
            w1h_j[0:2].transpose(1, 0, 2).reshape(P, 2 * H)
        )
        w1hb_j = np.ascontiguousarray(
            w1h_j[2:4].transpose(1, 0, 2).reshape(P, 2 * H)
        )
        w1la_j = np.ascontiguousarray(
            w1l_j[0:2].transpose(1, 0, 2).reshape(P, 2 * H)
        )
        w1lb_j = np.ascontiguousarray(
            w1l_j[2:4].transpose(1, 0, 2).reshape(P, 2 * H)
        )
        in_maps.append(
            {
                "hid": hid_j, "idxd": idxd,
                "w1ha": w1ha_j, "w1hb": w1hb_j,
                "w1la": w1la_j, "w1lb": w1lb_j,
            }
        )

    nc = _get_nc()
    res = run_bass_kernel_spmd(nc, in_maps, list(range(NCORES)), trace=TRACE)
    LAST_EXEC_NS = res.exec_time_ns

    parts = np.stack([res.results[j]["out"] for j in range(NCORES)])  # (8,32,1024)
    out1 = parts.sum(axis=0, dtype=np.float64) + b1.astype(np.float64)
    y = out1 @ W2.T.astype(np.float64) + b2.astype(np.float64)
    return np.ascontiguousarray(y.astype(np.float32))                 # (B, N)


# revision 20
# speedup vs baseline: 1.1264x; 1.0913x over previous
"""Trainium2 Bass kernel for nn_Expert (gather-span + 2-layer linear MLP).

Reference computation (B=32, L=4096, H=1024, N=4):
    idx      = pos + arange(N)                      # (B, N)
    gathered = hidden[b, idx[b, n], :]              # (B, N, H)
    x        = gathered.reshape(B, N*H)             # (B, 4096)
    out      = (x @ W1.T + b1) @ W2.T + b2          # (B, 4)

Sharding (8 cores): hidden sharded on the last dim (H) in 128-wide
slices; W1 sharded over the matching contraction columns (2MB/core).
The kernel computes the per-core contraction partial of x @ W1.T
(a (32, 1024) fp32 tile); the host sums the 8 partials and applies
the tiny second layer (1024->4) plus both biases during the reduction
it already performs (both are linear, so this is exact).

Precision trick: fp32 matmuls on the PE take two passes per streamed
column (~5.6 cyc/col); fp16 streams at 1 cyc/col. W1 and x are each
split into fp16 (hi, lo) parts and the product is assembled from
three fp16 passes accumulated exactly in fp32 PSUM:
    x@W1 ~ xh@wh + xl@wh + (xh*2^-11)@(wl*2^11)     (xl@wl ~ 2^-22, dropped)
W1's lo residuals (~7e-6) would be fp16-subnormal, so the host
pre-scales them by 2^11 and the matching hi stationary is down-scaled
on device; every operand stays in fp16 normal range and all passes
accumulate at natural scale. Measured ~7e-5 max rel err.

Latency engineering (the kernel is dominated by fixed latencies):
  - gather indices idx[b] = b*L + pos[b] are host-computed, shipped as
    a direct (32, 1) int32 DMA, first on the sync queue,
  - only W1's hi half (1MB) streams before the gather so HBM is quiet
    while the indirect gather's 32 2KB-span descriptors are in flight;
    the lo half is released only after the gather data lands (a 1-elem
    copy from xg into each lo tile forces the ordering), and the
    wl-consuming pass runs last so the late start is hidden,
  - the PE runs dummy transposes/matmuls sized to span the gather wait
    so the HAM activity monitor holds the clock at 2.4 GHz (a cold PE
    runs at 1.2 GHz and needs ~3.4us of sustained work to ramp),
  - stage 1 runs half-major (12 matmuls per PSUM half) so half 0's
    copy-out + store overlap half 1's matmuls; the final half's
    copy/store is split 3 ways across engines and queues.
"""

import numpy as np

from concourse import bass, bacc, mybir
from concourse.tile import TileContext
from concourse.bass_utils import run_bass_kernel_spmd
from concourse.masks import make_identity

B, L, H, N = 32, 4096, 1024, 4
NCORES = 8
HS = H // NCORES       # 128: per-core slice of the hidden dim
P = 128
HB = H // 2            # 512: psum bank width for stage 1
F32 = mybir.dt.float32
F16 = mybir.dt.float16
I32 = mybir.dt.int32
NWARM32 = 8            # cold fp32 dummy transposes (~0.42us each)
NWARM16 = 14           # fp16 dummy matmuls bridging to the gather
LO_SCALE = 2048.0      # 2^11: host upscale of W1's lo part

TRACE = False          # set True in test harnesses to profile
LAST_EXEC_NS = None

_nc_cache = None


def _build_nc():
    nc = bacc.Bacc(target_bir_lowering=False)
    hid = nc.declare_dram_parameter("hid", [B * L, HS], F32, isOutput=False)
    idxd = nc.declare_dram_parameter("idxd", [B, 1], I32, isOutput=False)
    # hi/lo W1, each as two (128, 2048) tiles: w1h_a = chunks 0,1 etc.
    w1ha = nc.declare_dram_parameter("w1ha", [P, 2 * H], F16, isOutput=False)
    w1hb = nc.declare_dram_parameter("w1hb", [P, 2 * H], F16, isOutput=False)
    w1la = nc.declare_dram_parameter("w1la", [P, 2 * H], F16, isOutput=False)
    w1lb = nc.declare_dram_parameter("w1lb", [P, 2 * H], F16, isOutput=False)
    out = nc.declare_dram_parameter("out", [B, H], F32, isOutput=True)

    with TileContext(nc) as tc:
        with (
            tc.tile_pool(name="sbuf", bufs=1) as spool,
            tc.tile_pool(name="ps1", bufs=2, space="PSUM") as ppool,
            tc.tile_pool(name="psx", bufs=1, space="PSUM") as xpool,
        ):
            # gather indices: direct (32, 1) int32 DMA, first on sync
            idx = spool.tile([B, 1], I32)
            nc.sync.dma_start(out=idx[:], in_=idxd[:])

            # W1 hi tiles stream immediately (done before gather flight)
            wha = spool.tile([P, 2 * H], F16, tag="wha", name="wha")
            nc.sync.dma_start(out=wha[:], in_=w1ha[:])
            whb = spool.tile([P, 2 * H], F16, tag="whb", name="whb")
            nc.scalar.dma_start(out=whb[:], in_=w1hb[:])

            ident = spool.tile([P, P], F32)
            make_identity(nc, ident[:])
            dummy16 = spool.tile([P, B], F16)
            nc.vector.memset(dummy16[:], 1.0)
            dummyS = spool.tile([P, HB], F16)
            nc.vector.memset(dummyS[:], 1.0)

            # indirect gather: xg[b, n*128+k] = hidden[b, pos[b]+n, k]
            # (one 2KB descriptor per batch: 4 consecutive rows of hid)
            xg = spool.tile([B, N * HS], F32)
            nc.gpsimd.indirect_dma_start(
                out=xg[:, :],
                out_offset=None,
                in_=hid[:],
                in_offset=bass.IndirectOffsetOnAxis(ap=idx[:, :1], axis=0),
                bounds_check=None,
            )

            # W1 lo tiles are released only after the gather data lands
            # (keeps HBM quiet during the gather's scattered reads); a
            # 1-elem GpSimd copy from xg into each tile forces the
            # ordering without occupying the DVE (which feeds the PE)
            wlt = []
            for n in range(N):
                t = spool.tile([P, H], F16, tag=f"wl{n}", name=f"wl{n}")
                nc.gpsimd.tensor_copy(out=t[:1, :2], in_=xg[:1, :2])
                src = w1la if n < 2 else w1lb
                eng = nc.sync if n % 2 == 0 else nc.scalar
                eng.dma_start(
                    out=t[:], in_=src[:, (n % 2) * H:(n % 2) * H + H]
                )
                wlt.append(t)

            def wh(n):
                t = wha if n < 2 else whb
                return t[:, (n % 2) * H:(n % 2) * H + H]

            def wl(n):
                return wlt[n][:, :]

            # PE warmup: cold fp32 transposes, then fp16 dummy matmuls
            # streaming the landed hi tile; spans the gather wait so the
            # HAM window stays hot into the real matmuls
            warm_ps = xpool.tile([P, P], F32, space="PSUM", tag="warm")
            for _ in range(NWARM32):
                nc.tensor.transpose(
                    out=warm_ps[:], in_=ident[:], identity=ident[:]
                )
            warm2_ps = xpool.tile([B, HB], F32, space="PSUM", tag="warm2")
            for _ in range(NWARM16):
                nc.tensor.matmul(
                    out=warm2_ps[:], lhsT=dummy16[:],
                    rhs=dummyS[:], start=True, stop=True,
                )

            # 4 strip transposes into one shared PSUM tile:
            # xt_ps[k, n*32+b] = xg[b, n*128+k]
            xt_ps = xpool.tile([P, P], F32, space="PSUM", tag="xt")
            for n in range(N):
                nc.tensor.transpose(
                    out=xt_ps[:, n * B:(n + 1) * B],
                    in_=xg[:, n * HS:(n + 1) * HS],
                    identity=ident[:B, :B],
                )

            # fp16 stationaries per strip: hi (DVE cast), hi*2^-11 (ACT
            # scaled copy), lo = x - hi (ACT fp32 round trip + DVE sub)
            xh, xhs, xl = [], [], []
            for n in range(N):
                h16 = spool.tile([P, B], F16, tag=f"xh{n}")
                nc.vector.tensor_copy(
                    out=h16[:], in_=xt_ps[:, n * B:(n + 1) * B]
                )
                xh.append(h16)
            h32s = []
            for n in range(N):
                h32 = spool.tile([P, B], F32, tag=f"xh32{n}")
                nc.scalar.copy(out=h32[:], in_=xh[n][:])
                h32s.append(h32)
            for n in range(N):
                l16 = spool.tile([P, B], F16, tag=f"xl{n}")
                nc.vector.tensor_tensor(
                    out=l16[:], in0=xt_ps[:, n * B:(n + 1) * B],
                    in1=h32s[n][:], op=mybir.AluOpType.subtract,
                )
                xl.append(l16)
            for n in range(N):
                hs16 = spool.tile([P, B], F16, tag=f"xhs{n}")
                nc.scalar.activation(
                    out=hs16[:], in_=xt_ps[:, n * B:(n + 1) * B],
                    func=mybir.ActivationFunctionType.Copy,
                    scale=1.0 / LO_SCALE,
                )
                xhs.append(hs16)

            # stage 1, half-major; the wl-consuming pass runs last so the
            # late-released lo tiles are off the critical path
            ps = [
                ppool.tile([B, HB], F32, space="PSUM", tag="ps1",
                           name=f"ps1_{i}")
                for i in range(2)
            ]
            passes = [(xh, wh), (xl, wh), (xhs, wl)]
            for half in range(2):
                for p, (stat, stream) in enumerate(passes):
                    for n in range(N):
                        nc.tensor.matmul(
                            out=ps[half][:],
                            lhsT=stat[n][:],
                            rhs=stream(n)[:, half * HB:(half + 1) * HB],
                            start=(p == 0 and n == 0),
                            stop=(p == 2 and n == N - 1),
                        )
                if half == 0:
                    o0 = spool.tile([B, HB], F32, tag="osb0")
                    nc.vector.tensor_copy(out=o0[:], in_=ps[0][:])
                    nc.sync.dma_start(out=out[:, :HB], in_=o0[:])
                else:
                    # final half: 3-way split across engines + queues,
                    # separate staging tiles so the copies don't pick up
                    # false whole-tile WAW deps across engines
                    HQ = HB // 2   # 256
                    HE = HB // 4   # 128
                    oA = spool.tile([B, HQ], F32, tag="osbA")
                    nc.vector.tensor_copy(out=oA[:], in_=ps[1][:, :HQ])
                    nc.sync.dma_start(out=out[:, HB:HB + HQ], in_=oA[:])
                    oB = spool.tile([B, HE], F32, tag="osbB")
                    nc.scalar.copy(out=oB[:], in_=ps[1][:, HQ:HQ + HE])
                    nc.scalar.dma_start(
                        out=out[:, HB + HQ:HB + HQ + HE], in_=oB[:]
                    )
                    oC = spool.tile([B, HE], F32, tag="osbC")
                    nc.vector.tensor_copy(out=oC[:], in_=ps[1][:, HQ + HE:])
                    nc.sync.dma_start(out=out[:, HB + HQ + HE:], in_=oC[:])

    nc.finalize()
    return nc


def _get_nc():
    global _nc_cache
    if _nc_cache is None:
        _nc_cache = _build_nc()
    return _nc_cache


def kernel(hidden, pos, W1, b1, W2, b2):
    global LAST_EXEC_NS
    hidden = np.asarray(hidden, dtype=np.float32)
    pos = np.asarray(pos)
    W1 = np.asarray(W1, dtype=np.float32)
    b1 = np.asarray(b1, dtype=np.float32)
    W2 = np.asarray(W2, dtype=np.float32)
    b2 = np.asarray(b2, dtype=np.float32)

    # gather row indices into hid (B*L, HS): idx[b] = b*L + pos[b]
    posv = pos.reshape(B).astype(np.int64)
    idxd = (np.arange(B, dtype=np.int64) * L + posv).reshape(B, 1).astype(
        np.int32
    )

    # W1 (H, N*H) -> per-core (N*P, H) fp16 hi/lo:
    #   w1t_j[n*P+k, o] = W1[o, n*H+j*HS+k]
    w1r = W1.reshape(H, N, NCORES, HS)                 # [o, n, j, k]

    in_maps = []
    for j in range(NCORES):
        hid_j = np.ascontiguousarray(
            hidden[:, :, j * HS:(j + 1) * HS]
        ).reshape(B * L, HS)
        w1t_j = w1r[:, :, j, :].transpose(1, 2, 0)     # [n, k, o]
        w1h_j = w1t_j.astype(np.float16)
        w1l_j = ((w1t_j - w1h_j.astype(np.float32)) * LO_SCALE).astype(
            np.float16
        )
        # (n, k, o) -> tiles (k, n*H+o) with n in {0,1} / {2,3}
        w1ha_j = np.ascontiguousarray(
            w1h_j[0:2].transpose(1, 0, 2).reshape(P, 2 * H)
        )
        w1hb_j = np.ascontiguousarray(
            w1h_j[2:4].transpose(1, 0, 2).reshape(P, 2 * H)
        )
        w1la_j = np.ascontiguousarray(
            w1l_j[0:2].transpose(1, 0, 2).reshape(P, 2 * H)
        )
        w1lb_j = np.ascontiguousarray(
            w1l_j[2:4].transpose(1, 0, 2).reshape(P, 2 * H)
        )
        in_maps.append(
            {
                "hid": hid_j, "idxd": idxd,
                "w1ha": w1ha_j, "w1hb": w1hb_j,
                "w1la": w1la_j, "w1lb": w1lb_j,
            }
        )

    nc = _get_nc()
    res = run_bass_kernel_spmd(nc, in_maps, list(range(NCORES)), trace=TRACE)
    LAST_EXEC_NS = res.exec_time_ns

    parts = np.stack([res.results[j]["out"] for j in range(NCORES)])  # (8,32,1024)
    out1 = parts.sum(axis=0, dtype=np.float64) + b1.astype(np.float64)
    y = out1 @ W2.T.astype(np.float64) + b2.astype(np.float64)
    return np.ascontiguousarray(y.astype(np.float32))                 # (B, N)
